# revision 19
# baseline (speedup 1.0000x reference)
"""Trainium2 Bass kernel for nn_HardQuadTripletSOSRLoss.

Sharding: 8 cores = 2 batches x 4 HW-shards (4096 grid cells each).
Device job (per core): scores = bf16(kp1_desc[b]) @ bf16(desc2f[b, shard])
-> fp32 PSUM, 16 units of [128 rows, 1024 cells], unit u = (window w=u//4,
row-tile t=u%4).  PSUM is split by lane parity: D-units (even u) use slots
0/1, A-units (odd u) slots 2/3, giving each engine a private double-buffer
(a PSUM bank may be read by only one engine at a time):
  - D-unit (DVE): tensor_reduce(max) groups of 8 cells -> fp8 group maxima.
  - A-unit (ACT): raw copy -> fp8.
Unit u2 is split half/half between DVE and ACT to balance the two chains
(DVE reduce ~1.10 ns/cell, ACT copy ~0.98 ns/cell).
Warm-up matmuls ramp the PE p-state while input DMAs are in flight.  All
candidate data is DMA'd to the host, which max-reduces the raw windows to
G=8, takes the top-K groups per row, rescores those cells exactly in fp32,
drops masked (neighbor) cells, and applies a certificate: rows where
hidden cells could reach the top-16 are recomputed exactly.  All other
stages (sampling, geometry, masks, SOS, loss) run on host, vectorized."""

import numpy as np
import ml_dtypes

import concourse.bass as bass
import concourse.mybir as mybir
from concourse import bacc
from concourse.bass_utils import run_bass_kernel_spmd

# ---- problem constants (hardcoded per contract) ----
B, N, C, H, W = 2, 512, 128, 128, 128
HW = H * W
GS = 8
NUM_NEG = 16
SOS_NEG = 8
MARGIN = 1.0
NSHARD = 4
SHW = HW // NSHARD          # 4096 cells per shard
UW = 1024                   # cells per unit
RT = N // 128               # 4 row tiles
CPB = 512                   # columns per matmul
GRP = 8                     # cells per group (contiguous)
NGR = UW // GRP             # 128 groups per unit
NWARM = 7                   # warm-up matmuls (PE p-state ramp)

ACOLS = 8 * UW + CPB        # 8704 raw cols (8 full A-units + half of u2)
DCOLS = 7 * NGR + NGR // 2  # 960 reduced cols

KSEL = 48                   # top-K groups rescored exactly per row
DELTA = 0.15                # certificate margin (fp8 out rounding + dot err)

F32 = mybir.dt.float32
BF16 = mybir.dt.bfloat16
F8 = mybir.dt.float8e4
BF = ml_dtypes.bfloat16
F8NP = ml_dtypes.float8_e4m3fn

_NC_CACHE = {}
LAST_RESULTS = None  # BassKernelResults of most recent device run (for test.py)


def _build_nc():
    nc = bacc.Bacc("TRN2", target_bir_lowering=False, debug=False, num_devices=8)

    lhsT = nc.dram_tensor("lhsT", [C, N], BF16, kind="ExternalInput")
    rhs = nc.dram_tensor("rhs", [C, SHW], BF16, kind="ExternalInput")
    # [A: 8704 raw fp8][D: 960 reduced fp8]
    cand = nc.dram_tensor("cand", [128, ACOLS + DCOLS], F8,
                          kind="ExternalOutput")

    # unit u: w=u//4 (col window), t=u%4 (row tile)
    # slots (1024 f32 = 4KB): even u -> slot (u//2)%2 in {0,1};
    #                         odd u  -> slot 2+((u-1)//2)%2 in {2,3}
    def pso(u):
        if u % 2 == 0:
            return ((u // 2) % 2) * UW
        return (2 + ((u - 1) // 2) % 2) * UW

    NG2 = NGR // 2
    # DVE jobs: (unit, col offset in unit, width, red_sb offset)
    DJOBS = [(0, 0, UW, 0), (2, 0, CPB, NGR)] + [
        (u, 0, UW, NGR + NG2 + i * NGR) for i, u in enumerate((4, 6, 8, 10, 12, 14))
    ]
    # ACT jobs: (unit, col offset in unit, width, cp_sb offset)
    AJOBS = [(1, 0, UW, 0), (2, CPB, CPB, UW), (3, 0, UW, UW + CPB)] + [
        (u, 0, UW, 2 * UW + CPB + i * UW) for i, u in enumerate((5, 7, 9, 11, 13, 15))
    ]
    DVE_JOB_OF = {}
    for i, (u, *_рest) in enumerate(DJOBS):
        DVE_JOB_OF.setdefault(u, []).append((u, i + 1))
    ACT_JOB_OF = {}
    for i, (u, *_rest) in enumerate(AJOBS):
        ACT_JOB_OF.setdefault(u, []).append((u, i + 1))

    with (
        nc.sbuf_tensor([C, N], BF16) as lhsT_sb,
        nc.sbuf_tensor([C, SHW], BF16) as rhs_sb,
        nc.sbuf_tensor([128, DCOLS], F8) as red_sb,
        nc.sbuf_tensor([128, ACOLS], F8) as cp_sb,
        nc.sbuf_tensor([128, CPB], BF16) as warm_sb,
        nc.psum_tensor([128, 4 * UW], F32) as ps,
        nc.semaphore() as dm_s,
        nc.semaphore() as dm_g,
        nc.semaphore() as dm_a,
        nc.semaphore() as mm_sem,
        nc.semaphore() as dve_sem,
        nc.semaphore() as act_sem,
        nc.semaphore() as out_sem,
        nc.semaphore() as warm_sem,
        nc.Block() as block,
    ):

        @block.sync
        def _(sync):
            # chunks c0a (256 cols), c0b on the sync queue (HWDGE)
            sync.dma_start(rhs_sb[:, :256], rhs[:, :256]).then_inc(dm_s, 16)
            sync.dma_start(
                rhs_sb[:, 256:UW], rhs[:, 256:UW]
            ).then_inc(dm_s, 16)
            # A chunks for jobs 4-5 (u5, u7) and 7-8 (u11, u13)
            sync.wait_ge(act_sem, 5)
            sync.dma_start(
                cand[:, 2 * UW + CPB : 4 * UW + CPB],
                cp_sb[:, 2 * UW + CPB : 4 * UW + CPB],
            ).then_inc(out_sem, 16)
            sync.wait_ge(act_sem, 8)
            sync.dma_start(
                cand[:, 4 * UW + CPB : 6 * UW + CPB],
                cp_sb[:, 4 * UW + CPB : 6 * UW + CPB],
            ).then_inc(out_sem, 16)
            sync.wait_ge(dve_sem, 8)
            sync.dma_start(
                cand[:, ACOLS + 5 * NGR + NGR // 2 :],
                red_sb[:, 5 * NGR + NGR // 2 :],
            ).then_inc(out_sem, 16)

        @block.gpsimd
        def _(gpsimd):
            # chunks c1, c3 on the gpsimd queue (SWDGE)
            gpsimd.dma_start(
                rhs_sb[:, UW : 2 * UW], rhs[:, UW : 2 * UW]
            ).then_inc(dm_g, 16)
            gpsimd.dma_start(
                rhs_sb[:, 3 * UW :], rhs[:, 3 * UW :]
            ).then_inc(dm_g, 16)
            # out chunks as lanes complete
            gpsimd.wait_ge(act_sem, 3)
            gpsimd.dma_start(
                cand[:, : 2 * UW + CPB], cp_sb[:, : 2 * UW + CPB]
            ).then_inc(out_sem, 16)
            gpsimd.wait_ge(dve_sem, 6)
            gpsimd.dma_start(
                cand[:, ACOLS : ACOLS + 5 * NGR + NGR // 2],
                red_sb[:, : 5 * NGR + NGR // 2],
            ).then_inc(out_sem, 16)

        @block.tensor
        def _(tensor):
            # warm-up: ramp PE p-state while input DMAs are in flight
            tensor.wait_ge(warm_sem, 1)
            for _ in range(NWARM):
                nc.tensor.matmul(
                    ps[:, 3 * UW : 3 * UW + CPB],
                    warm_sb[:, :128],
                    warm_sb[:, :CPB],
                    start=True,
                    stop=True,
                )
            for u in range(16):
                w, t = u // 4, u % 4
                # input chunk gating
                if u == 0:
                    tensor.wait_ge(dm_s, 16)   # c0a
                    tensor.wait_ge(dm_a, 16)   # lhsT
                elif u == 4:
                    tensor.wait_ge(dm_g, 16)   # c1
                elif u == 8:
                    tensor.wait_ge(dm_a, 32)   # c2
                elif u == 12:
                    tensor.wait_ge(dm_g, 32)   # c3
                # slot reuse gating: wait for the lane(s) that consumed u-4
                if u >= 4:
                    v = u - 4
                    for un, jn in DVE_JOB_OF.get(v, ()):
                        tensor.wait_ge(dve_sem, jn)
                    for un, jn in ACT_JOB_OF.get(v, ()):
                        tensor.wait_ge(act_sem, jn)
                off = pso(u)
                if u == 0:
                    # first 256 cols gated on c0a; rest on c0b
                    nc.tensor.matmul(
                        ps[:, off : off + 256],
                        lhsT_sb[:, :128],
                        rhs_sb[:, :256],
                        start=True,
                        stop=True,
                    )
                    tensor.wait_ge(dm_s, 32)   # c0b
                    nc.tensor.matmul(
                        ps[:, off + 256 : off + CPB],
                        lhsT_sb[:, :128],
                        rhs_sb[:, 256:CPB],
                        start=True,
                        stop=True,
                    )
                    mm = nc.tensor.matmul(
                        ps[:, off + CPB : off + UW],
                        lhsT_sb[:, :128],
                        rhs_sb[:, CPB:UW],
                        start=True,
                        stop=True,
                    )
                    mm.then_inc(mm_sem, 1)
                    continue
                for c in range(UW // CPB):
                    mm = nc.tensor.matmul(
                        ps[:, off + c * CPB : off + (c + 1) * CPB],
                        lhsT_sb[:, t * 128 : (t + 1) * 128],
                        rhs_sb[:, w * UW + c * CPB : w * UW + (c + 1) * CPB],
                        start=True,
                        stop=True,
                    )
                    if c == UW // CPB - 1:
                        mm.then_inc(mm_sem, 1)

        @block.vector
        def _(vector):
            nc.vector.memset(warm_sb[:, :], 0).then_inc(warm_sem, 1)
            for u, co, cw, ro in DJOBS:
                vector.wait_ge(mm_sem, u + 1)
                off = pso(u) + co
                nc.vector.tensor_reduce(
                    red_sb[:, ro : ro + cw // GRP],
                    ps[:, off : off + cw].rearrange("p (o k) -> p o k", k=GRP),
                    axis=mybir.AxisListType.X,
                    op=mybir.AluOpType.max,
                ).then_inc(dve_sem, 1)

        @block.scalar
        def _(scalar):
            # lhsT + chunk c2 on the scalar queue (HWDGE)
            scalar.dma_start(lhsT_sb[:], lhsT[:]).then_inc(dm_a, 16)
            scalar.dma_start(
                rhs_sb[:, 2 * UW : 3 * UW], rhs[:, 2 * UW : 3 * UW]
            ).then_inc(dm_a, 16)
            for u, co, cw, ao in AJOBS:
                scalar.wait_ge(mm_sem, u + 1)
                off = pso(u) + co
                nc.scalar.copy(
                    cp_sb[:, ao : ao + cw], ps[:, off : off + cw]
                ).then_inc(act_sem, 1)
            scalar.wait_ge(act_sem, 9)
            scalar.dma_start(
                cand[:, 6 * UW + CPB : ACOLS], cp_sb[:, 6 * UW + CPB :]
            ).then_inc(out_sem, 16)

    nc.compile()
    return nc


def _get_nc():
    if "nc" not in _NC_CACHE:
        _NC_CACHE["nc"] = _build_nc()
    return _NC_CACHE["nc"]


# ---------------- host-side helpers (all float32, mirror reference) ----------


def _sample_descriptors(desc2, kp):
    """Bilinear sample of desc2 (B,C,H,W) at image-space (y,x) kp, L2-normed."""
    b, c, h, w = desc2.shape
    f = np.float32
    y = np.clip(kp[..., 0] / f(GS) - f(0.5), f(0.0), f(h - 1.0)).astype(f)
    x = np.clip(kp[..., 1] / f(GS) - f(0.5), f(0.0), f(w - 1.0)).astype(f)
    y0 = np.clip(np.floor(y), 0, h - 2).astype(np.int64)
    x0 = np.clip(np.floor(x), 0, w - 2).astype(np.int64)
    wy = (y - y0.astype(f))[..., None]
    wx = (x - x0.astype(f))[..., None]
    dmap = desc2.transpose(0, 2, 3, 1).reshape(b, h * w, c)

    def g(yi, xi):
        idx = yi * w + xi
        return np.take_along_axis(dmap, idx[..., None], axis=1)

    v = (
        g(y0, x0) * (1 - wy) * (1 - wx)
        + g(y0, x0 + 1) * (1 - wy) * wx
        + g(y0 + 1, x0) * wy * (1 - wx)
        + g(y0 + 1, x0 + 1) * wy * wx
    )
    n = np.sqrt(np.sum(v * v, axis=-1, keepdims=True)).astype(f)
    return (v / (n + f(1e-8))).astype(f)


def _nearest4(pts):
    """Flat ids (..., 4) of the 4 nearest grid-cell centers, matching the
    reference's top_k over all HW cells (ties -> lower flat id)."""
    f = np.float32
    y = pts[..., 0]
    x = pts[..., 1]
    cy = np.clip(np.floor(y / f(GS)).astype(np.int64), 0, H - 1)
    cx = np.clip(np.floor(x / f(GS)).astype(np.int64), 0, W - 1)
    by = np.clip(cy - 2, 0, H - 5)
    bx = np.clip(cx - 2, 0, W - 5)
    offs = np.arange(5, dtype=np.int64)
    iy = by[..., None] + offs          # (..., 5)
    ix = bx[..., None] + offs
    cyc = (f(GS) * iy + f(GS / 2.0)).astype(f)
    cxc = (f(GS) * ix + f(GS / 2.0)).astype(f)
    dy = y[..., None] - cyc
    dx = x[..., None] - cxc
    d2 = (dy * dy)[..., :, None] + (dx * dx)[..., None, :]   # (..., 5, 5)
    ids = iy[..., :, None] * W + ix[..., None, :]
    d2 = d2.reshape(d2.shape[:-2] + (25,))
    ids = ids.reshape(ids.shape[:-2] + (25,))
    order = np.argsort(d2, axis=-1, kind="stable")[..., :4]
    return np.take_along_axis(ids, order, axis=-1)


def _warp(p, Hm):
    f = np.float32
    xy = p[..., ::-1]
    ph = np.concatenate([xy, np.ones_like(xy[..., :1])], axis=-1)
    wp = np.einsum("bij,bmj->bmi", Hm, ph).astype(f)
    wp = wp[..., :2] / (wp[..., 2:3] + f(1e-8))
    return wp[..., ::-1].astype(f)


def _centers(ids):
    f = np.float32
    yy = (ids // W).astype(f) * f(GS) + f(GS / 2.0)
    xx = (ids % W).astype(f) * f(GS) + f(GS / 2.0)
    return np.stack([yy, xx], axis=-1)


def _smallest8_ids(sim):
    """Indices of the 8 smallest values per row of sim (B,N,N), reference
    tie-break (lower index wins)."""
    part = np.argpartition(sim, SOS_NEG + 1, axis=-1)[..., : SOS_NEG + 2]
    vals = np.take_along_axis(sim, part, axis=-1)
    order = np.lexsort((part, vals), axis=-1)[..., :SOS_NEG]
    return np.take_along_axis(part, order, axis=-1)


def kernel(kp1, w_kp1, kp1_desc, desc2, homo12):
    global LAST_RESULTS
    import os

    f = np.float32
    kp1 = np.asarray(kp1, f)
    w_kp1 = np.asarray(w_kp1, f)
    kp1_desc = np.asarray(kp1_desc, f)
    desc2 = np.asarray(desc2, f)
    homo12 = np.asarray(homo12, f)

    # ---------------- host geometry / small tensors ----------------
    w_kp1_desc = _sample_descriptors(desc2, w_kp1)                  # (B,N,C)
    pos = f(2.0) - f(2.0) * np.einsum("bnc,bnc->bn", kp1_desc, w_kp1_desc)

    cell4 = _nearest4(kp1)                                          # (B,N,4)
    kp1_cells = _centers(cell4.reshape(B, 4 * N))                   # (B,4N,2)
    warped = _warp(kp1_cells, homo12)                               # (B,4N,2)
    wcc = _nearest4(warped)                                         # (B,4N,4)
    ids16 = wcc.reshape(B, N, 16)                                   # neigh cells
    cell4_w = _nearest4(w_kp1)                                      # (B,N,4)

    eqk = cell4[:, :, :, None, None] == cell4[:, None, None, :, :]
    kp1_mask = eqk.sum(axis=(2, 4)).astype(f)                       # (B,N,N)
    eqw = ids16[:, :, :, None, None] == cell4_w[:, None, None, :, :]
    w_kp1_mask = eqw.sum(axis=(2, 4)).astype(f)                     # (B,N,N)

    # ---------------- sos (entirely host) ----------------
    k_sim = (f(2.0) - f(2.0) * np.einsum("bnc,bmc->bnm", kp1_desc, kp1_desc)
             + kp1_mask * f(5.0))
    w_sim = (f(2.0) - f(2.0) * np.einsum("bnc,bmc->bnm", w_kp1_desc, w_kp1_desc)
             + w_kp1_mask * f(5.0))
    k_ids = _smallest8_ids(k_sim)                                   # (B,N,8)
    w_ids = _smallest8_ids(w_sim)
    kd = np.take_along_axis(
        kp1_desc, k_ids.reshape(B, N * 8)[:, :, None], axis=1
    ).reshape(B, N, 8, C)
    wd = np.take_along_axis(
        w_kp1_desc, w_ids.reshape(B, N * 8)[:, :, None], axis=1
    ).reshape(B, N, 8, C)
    a = f(2.0) - f(2.0) * np.einsum("bnc,bnkc->bnk", kp1_desc, kd)
    bb = f(2.0) - f(2.0) * np.einsum("bnc,bnkc->bnk", w_kp1_desc, wd)
    sv = (a - bb).astype(f)
    sos = np.mean(np.sqrt(np.sum(sv * sv, axis=-1))).astype(f)

    # ---------------- device run: group-max candidates ----------------
    nc = _get_nc()
    desc2_flat = desc2.reshape(B, C, HW)
    kp1_desc_bf = kp1_desc.astype(BF)
    desc2_bf = desc2_flat.astype(BF)
    in_maps = []
    for b in range(B):
        lhsT_b = np.ascontiguousarray(kp1_desc_bf[b].T)
        for s in range(NSHARD):
            in_maps.append(
                {
                    "lhsT": lhsT_b,
                    "rhs": np.ascontiguousarray(
                        desc2_bf[b][:, s * SHW : (s + 1) * SHW]
                    ),
                }
            )

    want_trace = bool(int(os.environ.get("KT_TRACE", "0")))
    try:
        res = run_bass_kernel_spmd(
            nc, in_maps, core_ids=list(range(8)), trace=want_trace
        )
    except ModuleNotFoundError:
        res = run_bass_kernel_spmd(nc, in_maps, core_ids=list(range(8)), trace=False)
    LAST_RESULTS = res
    results = res.results

    # ---------------- host merge: top-K groups, exact rescore ------------
    # Vals[b, n, s, w, o]: G=8 group max (device precision), cells
    # s*4096 + w*1024 + o*8 + k.
    NG2 = NGR // 2
    Vals = np.empty((B, N, NSHARD, 4, NGR), f)
    for ci in range(B * NSHARD):
        b, s = divmod(ci, NSHARD)
        c = np.asarray(results[ci]["cand"]).astype(f)
        red = c[:, ACOLS:]
        rr = c[:, :ACOLS].reshape(128, ACOLS // GRP, GRP).max(axis=2)
        # D jobs: u0 (w0,t0) red[0:128]; u2a (w0,t2 1st half) red[128:192];
        # then (u4,u6),(u8,u10),(u12,u14) = (w,t0),(w,t2) for w=1..3
        Vals[b, 0:128, s, 0] = red[:, 0:128]
        Vals[b, 256:384, s, 0, :NG2] = red[:, 128 : 128 + NG2]
        o0 = NGR + NG2
        for wi in (1, 2, 3):
            Vals[b, 0:128, s, wi] = red[:, o0 : o0 + NGR]
            Vals[b, 256:384, s, wi] = red[:, o0 + NGR : o0 + 2 * NGR]
            o0 += 2 * NGR
        # A jobs: u1 (w0,t1) rr[0:128]; u2b (w0,t2 2nd half) rr[128:192];
        # u3 (w0,t3) rr[192:320]; then u5..u15 odd
        Vals[b, 128:256, s, 0] = rr[:, 0:128]
        Vals[b, 256:384, s, 0, NG2:] = rr[:, 128 : 128 + NG2]
        Vals[b, 384:512, s, 0] = rr[:, 128 + NG2 : 128 + NG2 + NGR]
        o0 = 2 * NGR + NG2
        for u in (5, 7, 9, 11, 13, 15):
            wi, t = u // 4, u % 4
            Vals[b, t * 128 : (t + 1) * 128, s, wi] = rr[:, o0 : o0 + NGR]
            o0 += NGR

    V = Vals.reshape(B, N, NSHARD * 4 * NGR)                        # 2048 groups
    part = np.argpartition(-V, KSEL, axis=2)[:, :, :KSEL]           # (B, N, K)
    pv = np.take_along_axis(V, part, axis=2)
    vK = pv.min(axis=2)                                             # (B, N)

    base = part * GRP                                               # flat cell base
    cells = (base[..., None] + np.arange(GRP)).reshape(B, N, KSEL * GRP)

    hwdesc = desc2_flat.transpose(0, 2, 1)                          # (B, HW, C)
    gath = np.take_along_axis(
        hwdesc, cells.reshape(B, N * KSEL * GRP)[:, :, None], axis=1
    ).reshape(B, N, KSEL * GRP, C)
    ex = np.einsum("bnc,bnjc->bnj", kp1_desc, gath).astype(f)       # exact

    masked = (cells[..., None] == ids16[:, :, None, :]).any(axis=3)
    ex[masked] = -np.inf
    exs = -np.sort(-ex, axis=2)[:, :, :NUM_NEG]                     # (B, N, 16)
    t16 = exs[..., NUM_NEG - 1]

    repair = t16 < (vK + f(DELTA))                                  # (B, N)
    if repair.any():
        rb, rn = np.nonzero(repair)
        rows = np.einsum("jc,jhc->jh", kp1_desc[rb, rn], hwdesc[rb])
        for j in range(len(rb)):
            rows[j, ids16[rb[j], rn[j]]] = -np.inf
        rs = -np.sort(-rows, axis=1)[:, :NUM_NEG]
        exs[rb, rn] = rs

    neg = f(2.0) - f(2.0) * exs                                     # (B, N, 16)
    fos = np.mean(
        np.maximum(pos[..., None] - neg + f(MARGIN), f(0.0)) ** 2
    ).astype(f)

    return np.asarray(fos + sos, dtype=np.float32)
